# revision 1
# baseline (speedup 1.0000x reference)
"""Trainium2 Bass kernel for BatchPPRFeatures:
    out[i] = sum_k ppr_scores[i,k] * x[ppr_idx[i,k]]   (N=100000, K=32, D=128)

Strategy (8 NeuronCores, node-parallel):
- Shard output rows across 8 cores (12500 rows/core, padded to 13312 = 104
  tiles of 128). x (converted to fp16) is replicated to every core.
- The gather runs via gpsimd dma_gather (SWDGE) with int16 indices. Since
  int16 limits a gather call to <32768 table rows, x is split into 4 chunks
  of 25000 rows; each output tile's 4096 (i,k) entries are bucketed by chunk
  on the host into fixed-capacity segments (CAP slots, padded with index 0 /
  score 0), sorted ascending for DRAM locality.
- Gathered slots land as [slot%128 -> partition, slot//128 -> block]. The
  weighted reduction is BLOCKS_TILE accumulating PSUM matmuls per tile with
  host-prebuilt one-hot scatter matrices W[p, m] = score if slot(b,p) targets
  out-row m else 0, streamed from DRAM (contiguous DMA, overlaps the
  descriptor-bound gather for free).
- 4 SWDGE queues are used round-robin (one per chunk) - this parallelizes
  descriptor-ring drain and is ~3.5x faster than a single queue.
"""

import sys

sys.path.insert(0, "/opt/trn_rl_repo")

import numpy as np

N = 100000
D = 128
K = 32
N_CORES = 8
N_CHUNKS = 4
CHUNK = N // N_CHUNKS            # 25000 rows per chunk (int16-addressable)
ROWS_PER_CORE = N // N_CORES     # 12500
GROUP = 4                        # tiles per gather call group
CAP = 1280                       # slots per (tile, chunk) segment, mult of 128
BLOCKS_SEG = CAP // 128          # blocks per segment
TILES = 104                      # ceil(12500/128) padded to GROUP multiple
GROUPS = TILES // GROUP
ROWS_PAD = TILES * 128
BLOCKS_TILE = N_CHUNKS * BLOCKS_SEG          # blocks per tile
NBLOCKS = TILES * BLOCKS_TILE                # total W blocks per core
CALL_IDX = GROUP * CAP                       # indices per gather call
IDX_COLS = GROUPS * N_CHUNKS * (CALL_IDX // 16)

_prog_cache = {}


def _build_program():
    """Build + compile the (input-independent) SPMD Bass program."""
    if "nc" in _prog_cache:
        return _prog_cache["nc"]
    from concourse import bacc, mybir, tile

    F16 = mybir.dt.float16
    F32 = mybir.dt.float32
    I16 = mybir.dt.int16

    nc = bacc.Bacc(
        "TRN2",
        target_bir_lowering=False,
        debug=False,
        num_devices=N_CORES,
        num_swdge_queues=4,
    )
    x_d = nc.dram_tensor("x", [N, D], F16, kind="ExternalInput")
    idx_d = nc.dram_tensor("idx16", [128, IDX_COLS], I16, kind="ExternalInput")
    w_d = nc.dram_tensor("wmat", [128, NBLOCKS * 128], F16, kind="ExternalInput")
    out_d = nc.dram_tensor("out", [ROWS_PAD, D], F32, kind="ExternalOutput")

    with tile.TileContext(nc) as tc:
        with (
            tc.tile_pool(name="idxp", bufs=3) as idxp,
            tc.tile_pool(name="gp", bufs=2) as gpool,
            tc.tile_pool(name="wp", bufs=3) as wpool,
            tc.tile_pool(name="op", bufs=4) as opool,
            tc.tile_pool(name="ps", bufs=4, space="PSUM") as pspool,
        ):
            for g in range(GROUPS):
                idx_sb = idxp.tile([128, N_CHUNKS * CALL_IDX // 16], I16, tag="idx")
                nc.scalar.dma_start(
                    out=idx_sb[:],
                    in_=idx_d[
                        :,
                        g * N_CHUNKS * CALL_IDX // 16 : (g + 1)
                        * N_CHUNKS
                        * CALL_IDX
                        // 16,
                    ],
                )
                gs = []
                for c in range(N_CHUNKS):
                    g_sb = gpool.tile([128, GROUP * BLOCKS_SEG * D], F16, tag=f"g{c}")
                    nc.gpsimd.dma_gather(
                        out_ap=g_sb[:].rearrange("p (b d) -> p b d", d=D),
                        in_ap=x_d[c * CHUNK : (c + 1) * CHUNK, :],
                        idxs_ap=idx_sb[
                            :, c * CALL_IDX // 16 : (c + 1) * CALL_IDX // 16
                        ],
                        num_idxs=CALL_IDX,
                        num_idxs_reg=CALL_IDX,
                        elem_size=D,
                        single_packet=False,
                        queue_num=c,
                    )
                    gs.append(g_sb)

                for t in range(GROUP):
                    T = g * GROUP + t
                    w_sb = wpool.tile([128, BLOCKS_TILE * 128], F16, tag="w")
                    nc.sync.dma_start(
                        out=w_sb[:],
                        in_=w_d[
                            :, T * BLOCKS_TILE * 128 : (T + 1) * BLOCKS_TILE * 128
                        ],
                    )
                    ps = pspool.tile([128, D], F32, space="PSUM")
                    nb = 0
                    for c in range(N_CHUNKS):
                        for b in range(BLOCKS_SEG):
                            nc.tensor.matmul(
                                out=ps[:],
                                lhsT=w_sb[
                                    :,
                                    (c * BLOCKS_SEG + b) * 128 : (c * BLOCKS_SEG + b + 1)
                                    * 128,
                                ],
                                rhs=gs[c][
                                    :,
                                    (t * BLOCKS_SEG + b) * D : (t * BLOCKS_SEG + b + 1)
                                    * D,
                                ],
                                start=(nb == 0),
                                stop=(nb == BLOCKS_TILE - 1),
                            )
                            nb += 1
                    o_sb = opool.tile([128, D], F32, tag="o")
                    nc.scalar.copy(out=o_sb[:], in_=ps[:])
                    nc.sync.dma_start(
                        out=out_d[T * 128 : (T + 1) * 128, :], in_=o_sb[:]
                    )

    nc.compile()
    _prog_cache["nc"] = nc
    return nc


def _prep_core_inputs(idx_core, sc_core):
    """Bucket one core's (padded) indices by chunk into fixed-cap segments.

    idx_core: [ROWS_PAD, K] int64, sc_core: [ROWS_PAD, K] float32.
    Returns (idx16 [128, IDX_COLS] int16, wmat [128, NBLOCKS*128] f16).

    W layout: for tile T, chunk c, block b (gb = (T*N_CHUNKS+c)*BLOCKS_SEG+b),
    lane p (= slot b*128+p of segment (T,c)):
        wmat[p, gb*128 + m] = score  if the slot's entry targets out-row m.
    """
    seg_idx = np.zeros((TILES, N_CHUNKS, CAP), dtype=np.int16)
    # target row / score per slot
    seg_tc = np.zeros((TILES, N_CHUNKS, CAP), dtype=np.int64)
    seg_sp = np.zeros((TILES, N_CHUNKS, CAP), dtype=np.float16)

    idx_t = idx_core.reshape(TILES, 128 * K)
    sc_t = sc_core.reshape(TILES, 128 * K)
    chunk_t = idx_t // CHUNK
    p_of_e = np.arange(128 * K) // K  # target out-row of entry

    for T in range(TILES):
        ch = chunk_t[T]
        order = np.argsort(ch * N + idx_t[T], kind="stable")
        ch_s = ch[order]
        bounds = np.searchsorted(ch_s, np.arange(N_CHUNKS + 1))
        for c in range(N_CHUNKS):
            sel = order[bounds[c] : bounds[c + 1]]
            n = len(sel)
            if n > CAP:
                raise OverflowError(
                    f"segment overflow tile={T} chunk={c} n={n} > CAP={CAP}"
                )
            seg_idx[T, c, :n] = (idx_t[T, sel] - c * CHUNK).astype(np.int16)
            seg_tc[T, c, :n] = p_of_e[sel]
            seg_sp[T, c, :n] = sc_t[T, sel]

    # gather call lists: per (g, c) concat over t -> [CALL_IDX]
    calls = (
        seg_idx.reshape(GROUPS, GROUP, N_CHUNKS, CAP)
        .transpose(0, 2, 1, 3)
        .reshape(GROUPS * N_CHUNKS, CALL_IDX)
    )
    wrapped = calls.reshape(GROUPS * N_CHUNKS, CALL_IDX // 16, 16).transpose(0, 2, 1)
    idx16 = np.tile(
        wrapped.transpose(1, 0, 2).reshape(16, IDX_COLS), (8, 1)
    ).astype(np.int16)

    # W matrices: [128 lanes, NBLOCKS, 128 out-rows]
    wmat = np.zeros((128, NBLOCKS, 128), dtype=np.float16)
    # block index gb for (T, c, b); slot lane p
    tc_blocks = seg_tc.reshape(TILES, N_CHUNKS, BLOCKS_SEG, 128).transpose(
        3, 0, 1, 2
    ).reshape(128, NBLOCKS)
    sp_blocks = seg_sp.reshape(TILES, N_CHUNKS, BLOCKS_SEG, 128).transpose(
        3, 0, 1, 2
    ).reshape(128, NBLOCKS)
    pp, bb = np.meshgrid(np.arange(128), np.arange(NBLOCKS), indexing="ij")
    wmat[pp.ravel(), bb.ravel(), tc_blocks.ravel()] = sp_blocks.ravel()

    return (
        np.ascontiguousarray(idx16),
        np.ascontiguousarray(wmat.reshape(128, NBLOCKS * 128)),
    )


def make_in_maps(x, ppr_idx, ppr_scores):
    x16 = np.asarray(x).astype(np.float16)
    ppr_idx = np.asarray(ppr_idx)
    ppr_scores = np.asarray(ppr_scores)

    idx_pad = np.zeros((N_CORES, ROWS_PAD, K), dtype=np.int64)
    sc_pad = np.zeros((N_CORES, ROWS_PAD, K), dtype=np.float32)
    # spread zero-weight padding rows' indices across chunks so no
    # per-(tile, chunk) segment overflows its fixed capacity
    idx_pad[:, ROWS_PER_CORE:] = (np.arange(K) % N_CHUNKS) * CHUNK
    idx_pad[:, :ROWS_PER_CORE] = ppr_idx.reshape(N_CORES, ROWS_PER_CORE, K)
    sc_pad[:, :ROWS_PER_CORE] = ppr_scores.reshape(N_CORES, ROWS_PER_CORE, K)

    in_maps = []
    for c in range(N_CORES):
        idx16, wmat = _prep_core_inputs(idx_pad[c], sc_pad[c])
        in_maps.append({"x": x16, "idx16": idx16, "wmat": wmat})
    return in_maps


def kernel(x, ppr_idx, ppr_scores):
    from concourse.bass_utils import run_bass_kernel_spmd

    nc = _build_program()
    in_maps = make_in_maps(x, ppr_idx, ppr_scores)
    res = run_bass_kernel_spmd(nc, in_maps, core_ids=list(range(N_CORES)))
    out = np.concatenate(
        [res.results[c]["out"][:ROWS_PER_CORE] for c in range(N_CORES)], axis=0
    )
    return out.astype(np.float32)



# revision 6
# speedup vs baseline: 1.9030x; 1.9030x over previous
"""Trainium2 Bass kernel for BatchPPRFeatures:
    out[i] = sum_k ppr_scores[i,k] * x[ppr_idx[i,k]]   (N=100000, K=32, D=128)

Strategy (8 NeuronCores, node-parallel):
- Shard output rows across 8 cores (12500 rows/core, padded to 13312 = 104
  tiles of 128). x (converted to fp16) is replicated to every core.
- The gather runs via gpsimd dma_gather (SWDGE) with int16 indices. Since
  int16 limits a gather call to <32768 table rows, x is split into 4 chunks
  of 25000 rows; each output tile's 4096 (i,k) entries are bucketed by chunk
  on the host into fixed-capacity segments (CAP slots), deduplicated (equal
  indices share one slot) and sorted ascending for DRAM locality. Segments
  that overflow CAP spill their excess entries to a host-side numpy
  correction (rare: CAP sits at the dedup'd segment mean, ~0.4% of entries).
- Gathered slots land as [slot%128 -> partition, slot//128 -> block]. The
  weighted reduction is BLOCKS_TILE accumulating PSUM matmuls per tile with
  host-prebuilt one-hot scatter matrices W[p, m] = sum of scores of entries
  in slot (b,p) targeting out-row m, streamed from DRAM (contiguous DMA on
  the Vector engine's HWDGE queue, overlapping the descriptor-bound gather).
- 4 SWDGE queues are used round-robin (one per chunk) to parallelize
  descriptor-ring drain; matmuls are interleaved per-chunk so PE work for a
  tile starts as soon as that chunk's slots land.
"""

import sys

sys.path.insert(0, "/opt/trn_rl_repo")

import numpy as np

N = 100000
D = 128
K = 32
N_CORES = 8
N_CHUNKS = 4
CHUNK = N // N_CHUNKS            # 25000 rows per chunk (int16-addressable)
ROWS_PER_CORE = N // N_CORES     # 12500
GROUP = 4                        # tiles per gather call group
CAP = 1024                       # slots per (tile, chunk) segment, mult of 128
BLOCKS_SEG = CAP // 128          # blocks per segment
TILES = 104                      # ceil(12500/128) padded to GROUP multiple
GROUPS = TILES // GROUP
ROWS_PAD = TILES * 128
BLOCKS_TILE = N_CHUNKS * BLOCKS_SEG          # blocks per tile
NBLOCKS = TILES * BLOCKS_TILE                # total W blocks per core
CALL_IDX = GROUP * CAP                       # indices per gather call
IDX_COLS = GROUPS * N_CHUNKS * (CALL_IDX // 16)

_prog_cache = {}


def _build_program():
    """Build + compile the (input-independent) SPMD Bass program."""
    if "nc" in _prog_cache:
        return _prog_cache["nc"]
    from concourse import bacc, mybir, tile

    F16 = mybir.dt.float16
    F32 = mybir.dt.float32
    I16 = mybir.dt.int16

    nc = bacc.Bacc(
        "TRN2",
        target_bir_lowering=False,
        debug=False,
        num_devices=N_CORES,
        num_swdge_queues=4,
    )
    x_d = nc.dram_tensor("x", [N, D], F16, kind="ExternalInput")
    idx_d = nc.dram_tensor("idx16", [128, IDX_COLS], I16, kind="ExternalInput")
    w_d = nc.dram_tensor("wmat", [128, NBLOCKS * 128], F16, kind="ExternalInput")
    out_d = nc.dram_tensor("out", [ROWS_PAD, D], F32, kind="ExternalOutput")

    with tile.TileContext(nc) as tc:
        with (
            tc.tile_pool(name="idxp", bufs=3) as idxp,
            tc.tile_pool(name="gp", bufs=3) as gpool,
            tc.tile_pool(name="wp", bufs=3) as wpool,
            tc.tile_pool(name="op", bufs=4) as opool,
            tc.tile_pool(name="ps", bufs=2, space="PSUM") as pspool,
        ):
            for g in range(GROUPS):
                idx_sb = idxp.tile([128, N_CHUNKS * CALL_IDX // 16], I16, tag="idx")
                nc.scalar.dma_start(
                    out=idx_sb[:],
                    in_=idx_d[
                        :,
                        g * N_CHUNKS * CALL_IDX // 16 : (g + 1)
                        * N_CHUNKS
                        * CALL_IDX
                        // 16,
                    ],
                )
                gs = []
                for c in range(N_CHUNKS):
                    g_sb = gpool.tile([128, GROUP * BLOCKS_SEG * D], F16, tag=f"g{c}")
                    nc.gpsimd.dma_gather(
                        out_ap=g_sb[:].rearrange("p (b d) -> p b d", d=D),
                        in_ap=x_d[c * CHUNK : (c + 1) * CHUNK, :],
                        idxs_ap=idx_sb[
                            :, c * CALL_IDX // 16 : (c + 1) * CALL_IDX // 16
                        ],
                        num_idxs=CALL_IDX,
                        num_idxs_reg=CALL_IDX,
                        elem_size=D,
                        single_packet=False,
                        queue_num=c,
                    )
                    gs.append(g_sb)

                ws = []
                ps_tiles = []
                for t in range(GROUP):
                    T = g * GROUP + t
                    w_sb = wpool.tile([128, BLOCKS_TILE * 128], F16, tag=f"w{t}")
                    nc.sync.dma_start(
                        out=w_sb[:],
                        in_=w_d[
                            :, T * BLOCKS_TILE * 128 : (T + 1) * BLOCKS_TILE * 128
                        ],
                    )
                    ws.append(w_sb)
                    ps = pspool.tile([128, D], F32, space="PSUM", tag=f"ps{t}")
                    ps_tiles.append(ps)

                # per-chunk interleave: tile t's matmuls for chunk c start as
                # soon as that chunk's gather lands
                for c in range(N_CHUNKS):
                    for t in range(GROUP):
                        for b in range(BLOCKS_SEG):
                            nc.tensor.matmul(
                                out=ps_tiles[t][:],
                                lhsT=ws[t][
                                    :,
                                    (c * BLOCKS_SEG + b) * 128 : (c * BLOCKS_SEG + b + 1)
                                    * 128,
                                ],
                                rhs=gs[c][
                                    :,
                                    (t * BLOCKS_SEG + b) * D : (t * BLOCKS_SEG + b + 1)
                                    * D,
                                ],
                                start=(c == 0 and b == 0),
                                stop=(c == N_CHUNKS - 1 and b == BLOCKS_SEG - 1),
                            )
                for t in range(GROUP):
                    T = g * GROUP + t
                    o_sb = opool.tile([128, D], F32, tag="o")
                    nc.scalar.copy(out=o_sb[:], in_=ps_tiles[t][:])
                    nc.scalar.dma_start(
                        out=out_d[T * 128 : (T + 1) * 128, :], in_=o_sb[:]
                    )

    nc.compile()
    _prog_cache["nc"] = nc
    return nc


def _prep_core_inputs(idx_core, sc_core):
    """Bucket one core's (padded) indices by chunk into fixed-cap segments.

    idx_core: [ROWS_PAD, K] int64, sc_core: [ROWS_PAD, K] float32.
    Returns (idx16 [128, IDX_COLS] int16, wmat [128, NBLOCKS*128] f16,
             spill list of (local_row, src_row, score)).

    Equal indices within a (tile, chunk) segment share one gather slot; the
    W matrix accumulates all their scores. Segments whose distinct-index
    count exceeds CAP spill the excess entries to the host-side correction.

    W layout: for tile T, chunk c, block b (gb = (T*N_CHUNKS+c)*BLOCKS_SEG+b),
    lane p (= slot b*128+p of segment (T,c)):
        wmat[p, gb*128 + m] = sum of scores of the slot's entries
                              targeting out-row m.
    """
    seg_idx = np.zeros((TILES, N_CHUNKS, CAP), dtype=np.int16)
    # per-entry slot assignment: (tile, chunk, slot, target row, score)
    w_lane = []
    w_block = []
    w_tgt = []
    w_score = []
    spill = []

    idx_t = idx_core.reshape(TILES, 128 * K)
    sc_t = sc_core.reshape(TILES, 128 * K)
    chunk_t = idx_t // CHUNK
    p_of_e = np.arange(128 * K) // K  # target out-row of entry

    for T in range(TILES):
        ch = chunk_t[T]
        for c in range(N_CHUNKS):
            sel = np.nonzero(ch == c)[0]
            vals = idx_t[T, sel]
            uniq, inv = np.unique(vals, return_inverse=True)
            n_u = len(uniq)
            if n_u > CAP:
                keep = inv < CAP
                for e, s_inv in zip(sel[~keep], inv[~keep]):
                    spill.append((T * 128 + p_of_e[e], idx_t[T, e], sc_t[T, e]))
                sel = sel[keep]
                inv = inv[keep]
                uniq = uniq[:CAP]
                n_u = CAP
            seg_idx[T, c, :n_u] = (uniq - c * CHUNK).astype(np.int16)
            slot = inv  # slot within segment
            w_lane.append(slot % 128)
            w_block.append((T * N_CHUNKS + c) * BLOCKS_SEG + slot // 128)
            w_tgt.append(p_of_e[sel])
            w_score.append(sc_t[T, sel])

    wmat = np.zeros((128, NBLOCKS, 128), dtype=np.float32)
    np.add.at(
        wmat,
        (np.concatenate(w_lane), np.concatenate(w_block), np.concatenate(w_tgt)),
        np.concatenate(w_score),
    )

    # gather call lists: per (g, c) concat over t -> [CALL_IDX]
    calls = (
        seg_idx.reshape(GROUPS, GROUP, N_CHUNKS, CAP)
        .transpose(0, 2, 1, 3)
        .reshape(GROUPS * N_CHUNKS, CALL_IDX)
    )
    wrapped = calls.reshape(GROUPS * N_CHUNKS, CALL_IDX // 16, 16).transpose(0, 2, 1)
    idx16 = np.tile(
        wrapped.transpose(1, 0, 2).reshape(16, IDX_COLS), (8, 1)
    ).astype(np.int16)

    return (
        np.ascontiguousarray(idx16),
        np.ascontiguousarray(wmat.astype(np.float16).reshape(128, NBLOCKS * 128)),
        spill,
    )


def _prep_all(x, ppr_idx, ppr_scores):
    x16 = np.asarray(x).astype(np.float16)
    ppr_idx = np.asarray(ppr_idx)
    ppr_scores = np.asarray(ppr_scores)

    idx_pad = np.zeros((N_CORES, ROWS_PAD, K), dtype=np.int64)
    sc_pad = np.zeros((N_CORES, ROWS_PAD, K), dtype=np.float32)
    # spread zero-weight padding rows' indices across chunks; they dedup to
    # one slot per segment
    idx_pad[:, ROWS_PER_CORE:] = (np.arange(K) % N_CHUNKS) * CHUNK
    idx_pad[:, :ROWS_PER_CORE] = ppr_idx.reshape(N_CORES, ROWS_PER_CORE, K)
    sc_pad[:, :ROWS_PER_CORE] = ppr_scores.reshape(N_CORES, ROWS_PER_CORE, K)

    in_maps = []
    spills = []
    for c in range(N_CORES):
        idx16, wmat, spill = _prep_core_inputs(idx_pad[c], sc_pad[c])
        in_maps.append({"x": x16, "idx16": idx16, "wmat": wmat})
        spills.append(spill)
    return in_maps, spills


def make_in_maps(x, ppr_idx, ppr_scores):
    in_maps, _ = _prep_all(x, ppr_idx, ppr_scores)
    return in_maps


def kernel(x, ppr_idx, ppr_scores):
    from concourse.bass_utils import run_bass_kernel_spmd

    nc = _build_program()
    x = np.asarray(x)
    in_maps, spills = _prep_all(x, ppr_idx, ppr_scores)
    res = run_bass_kernel_spmd(nc, in_maps, core_ids=list(range(N_CORES)))
    out = np.concatenate(
        [res.results[c]["out"][:ROWS_PER_CORE] for c in range(N_CORES)], axis=0
    ).astype(np.float32)
    # host-side correction for rare segment-capacity spills
    for c in range(N_CORES):
        for local_row, src_row, score in spills[c]:
            if local_row < ROWS_PER_CORE:
                out[c * ROWS_PER_CORE + local_row] += score * x[src_row]
    return out


# revision 8
# speedup vs baseline: 11.1440x; 5.8559x over previous
"""Trainium2 Bass kernel for BatchPPRFeatures:
    out[i] = sum_k ppr_scores[i,k] * x[ppr_idx[i,k]]   (N=100000, K=32, D=128)

Strategy (8 NeuronCores, node-parallel):
- Shard output rows across 8 cores (12500 rows/core, padded to 13312 = 104
  tiles of 128). x (converted to fp16) is replicated to every core.
- The gather runs via gpsimd dma_gather (SWDGE) with int16 indices. Since
  int16 limits a gather call to <32768 table rows, x is split into 4 chunks
  of 25000 rows; each output tile's 4096 (i,k) entries are bucketed by chunk
  on the host into fixed-capacity segments (CAP slots), deduplicated (equal
  indices share one slot) and sorted ascending for DRAM locality. Segments
  that overflow CAP spill their excess entries to a host-side numpy
  correction (rare: CAP sits at the dedup'd segment mean, ~0.4% of entries).
- Gathered slots land as [slot%128 -> partition, slot//128 -> block]. The
  weighted reduction is BLOCKS_TILE accumulating PSUM matmuls per tile with
  host-prebuilt one-hot scatter matrices W[p, m] = sum of scores of entries
  in slot (b,p) targeting out-row m, streamed from DRAM (contiguous DMA on
  the Vector engine's HWDGE queue, overlapping the descriptor-bound gather).
- 4 SWDGE queues are used round-robin (one per chunk) to parallelize
  descriptor-ring drain; matmuls are interleaved per-chunk so PE work for a
  tile starts as soon as that chunk's slots land.
"""

import sys

sys.path.insert(0, "/opt/trn_rl_repo")

import numpy as np

N = 100000
D = 128
K = 32
N_CORES = 8
N_CHUNKS = 4
CHUNK = N // N_CHUNKS            # 25000 rows per chunk (int16-addressable)
ROWS_PER_CORE = N // N_CORES     # 12500
GROUP = 4                        # tiles per gather call group
CAP = 1024                       # slots per (tile, chunk) segment, mult of 128
BLOCKS_SEG = CAP // 128          # blocks per segment
TILES = 104                      # ceil(12500/128) padded to GROUP multiple
GROUPS = TILES // GROUP
ROWS_PAD = TILES * 128
BLOCKS_TILE = N_CHUNKS * BLOCKS_SEG          # blocks per tile
NBLOCKS = TILES * BLOCKS_TILE                # total W blocks per core
CALL_IDX = GROUP * CAP                       # indices per gather call
IDX_COLS = GROUPS * N_CHUNKS * (CALL_IDX // 16)

_prog_cache = {}


def _build_program():
    """Build + compile the (input-independent) SPMD Bass program."""
    if "nc" in _prog_cache:
        return _prog_cache["nc"]
    from concourse import bacc, mybir, tile

    F16 = mybir.dt.float16
    F32 = mybir.dt.float32
    I16 = mybir.dt.int16

    nc = bacc.Bacc(
        "TRN2",
        target_bir_lowering=False,
        debug=False,
        num_devices=N_CORES,
        num_swdge_queues=4,
    )
    x_d = nc.dram_tensor("x", [N, D], F16, kind="ExternalInput")
    idx_d = nc.dram_tensor("idx16", [128, IDX_COLS], I16, kind="ExternalInput")
    w_d = nc.dram_tensor("wmat", [128, NBLOCKS * 128], F16, kind="ExternalInput")
    out_d = nc.dram_tensor("out", [ROWS_PAD, D], F32, kind="ExternalOutput")

    with tile.TileContext(nc) as tc:
        with (
            tc.tile_pool(name="idxp", bufs=3) as idxp,
            tc.tile_pool(name="gp", bufs=3) as gpool,
            tc.tile_pool(name="wp", bufs=3) as wpool,
            tc.tile_pool(name="op", bufs=4) as opool,
            tc.tile_pool(name="ps", bufs=2, space="PSUM") as pspool,
        ):
            dma_sems = [nc.alloc_semaphore(f"swdge_dma_{c}") for c in range(N_CHUNKS)]
            for g in range(GROUPS):
                idx_sb = idxp.tile([128, N_CHUNKS * CALL_IDX // 16], I16, tag="idx")
                nc.scalar.dma_start(
                    out=idx_sb[:],
                    in_=idx_d[
                        :,
                        g * N_CHUNKS * CALL_IDX // 16 : (g + 1)
                        * N_CHUNKS
                        * CALL_IDX
                        // 16,
                    ],
                )
                gs = []
                for c in range(N_CHUNKS):
                    g_sb = gpool.tile([128, GROUP * BLOCKS_SEG * D], F16, tag=f"g{c}")
                    # prepare_only + trigger decouples Q7 descriptor
                    # generation from the DMA drain: descgen for group g+1
                    # runs while group g's descriptors are still in flight
                    nc.gpsimd.dma_gather(
                        out_ap=g_sb[:].rearrange("p (b d) -> p b d", d=D),
                        in_ap=x_d[c * CHUNK : (c + 1) * CHUNK, :],
                        idxs_ap=idx_sb[
                            :, c * CALL_IDX // 16 : (c + 1) * CALL_IDX // 16
                        ],
                        num_idxs=CALL_IDX,
                        num_idxs_reg=CALL_IDX,
                        elem_size=D,
                        single_packet=False,
                        queue_num=c,
                        prepare_only=True,
                        sem=dma_sems[c],
                    )
                    nc.gpsimd.trigger_dma(count=None, queue_num=c)
                    gs.append(g_sb)

                ws = []
                ps_tiles = []
                for t in range(GROUP):
                    T = g * GROUP + t
                    w_sb = wpool.tile([128, BLOCKS_TILE * 128], F16, tag=f"w{t}")
                    nc.sync.dma_start(
                        out=w_sb[:],
                        in_=w_d[
                            :, T * BLOCKS_TILE * 128 : (T + 1) * BLOCKS_TILE * 128
                        ],
                    )
                    ws.append(w_sb)
                    ps = pspool.tile([128, D], F32, space="PSUM", tag=f"ps{t}")
                    ps_tiles.append(ps)

                # per-chunk interleave: tile t's matmuls for chunk c start as
                # soon as that chunk's gather lands
                for c in range(N_CHUNKS):
                    for t in range(GROUP):
                        for b in range(BLOCKS_SEG):
                            nc.tensor.matmul(
                                out=ps_tiles[t][:],
                                lhsT=ws[t][
                                    :,
                                    (c * BLOCKS_SEG + b) * 128 : (c * BLOCKS_SEG + b + 1)
                                    * 128,
                                ],
                                rhs=gs[c][
                                    :,
                                    (t * BLOCKS_SEG + b) * D : (t * BLOCKS_SEG + b + 1)
                                    * D,
                                ],
                                start=(c == 0 and b == 0),
                                stop=(c == N_CHUNKS - 1 and b == BLOCKS_SEG - 1),
                            )
                for t in range(GROUP):
                    T = g * GROUP + t
                    o_sb = opool.tile([128, D], F32, tag="o")
                    nc.scalar.copy(out=o_sb[:], in_=ps_tiles[t][:])
                    nc.scalar.dma_start(
                        out=out_d[T * 128 : (T + 1) * 128, :], in_=o_sb[:]
                    )

    nc.compile()
    _prog_cache["nc"] = nc
    return nc


def _prep_core_inputs(idx_core, sc_core):
    """Bucket one core's (padded) indices by chunk into fixed-cap segments.

    idx_core: [ROWS_PAD, K] int64, sc_core: [ROWS_PAD, K] float32.
    Returns (idx16 [128, IDX_COLS] int16, wmat [128, NBLOCKS*128] f16,
             spill list of (local_row, src_row, score)).

    Equal indices within a (tile, chunk) segment share one gather slot; the
    W matrix accumulates all their scores. Segments whose distinct-index
    count exceeds CAP spill the excess entries to the host-side correction.

    W layout: for tile T, chunk c, block b (gb = (T*N_CHUNKS+c)*BLOCKS_SEG+b),
    lane p (= slot b*128+p of segment (T,c)):
        wmat[p, gb*128 + m] = sum of scores of the slot's entries
                              targeting out-row m.
    """
    seg_idx = np.zeros((TILES, N_CHUNKS, CAP), dtype=np.int16)
    # per-entry slot assignment: (tile, chunk, slot, target row, score)
    w_lane = []
    w_block = []
    w_tgt = []
    w_score = []
    spill = []

    idx_t = idx_core.reshape(TILES, 128 * K)
    sc_t = sc_core.reshape(TILES, 128 * K)
    chunk_t = idx_t // CHUNK
    p_of_e = np.arange(128 * K) // K  # target out-row of entry

    for T in range(TILES):
        ch = chunk_t[T]
        for c in range(N_CHUNKS):
            sel = np.nonzero(ch == c)[0]
            vals = idx_t[T, sel]
            uniq, inv = np.unique(vals, return_inverse=True)
            n_u = len(uniq)
            if n_u > CAP:
                keep = inv < CAP
                for e, s_inv in zip(sel[~keep], inv[~keep]):
                    spill.append((T * 128 + p_of_e[e], idx_t[T, e], sc_t[T, e]))
                sel = sel[keep]
                inv = inv[keep]
                uniq = uniq[:CAP]
                n_u = CAP
            seg_idx[T, c, :n_u] = (uniq - c * CHUNK).astype(np.int16)
            slot = inv  # slot within segment
            w_lane.append(slot % 128)
            w_block.append((T * N_CHUNKS + c) * BLOCKS_SEG + slot // 128)
            w_tgt.append(p_of_e[sel])
            w_score.append(sc_t[T, sel])

    wmat = np.zeros((128, NBLOCKS, 128), dtype=np.float32)
    np.add.at(
        wmat,
        (np.concatenate(w_lane), np.concatenate(w_block), np.concatenate(w_tgt)),
        np.concatenate(w_score),
    )

    # gather call lists: per (g, c) concat over t -> [CALL_IDX]
    calls = (
        seg_idx.reshape(GROUPS, GROUP, N_CHUNKS, CAP)
        .transpose(0, 2, 1, 3)
        .reshape(GROUPS * N_CHUNKS, CALL_IDX)
    )
    wrapped = calls.reshape(GROUPS * N_CHUNKS, CALL_IDX // 16, 16).transpose(0, 2, 1)
    idx16 = np.tile(
        wrapped.transpose(1, 0, 2).reshape(16, IDX_COLS), (8, 1)
    ).astype(np.int16)

    return (
        np.ascontiguousarray(idx16),
        np.ascontiguousarray(wmat.astype(np.float16).reshape(128, NBLOCKS * 128)),
        spill,
    )


def _prep_all(x, ppr_idx, ppr_scores):
    x16 = np.asarray(x).astype(np.float16)
    ppr_idx = np.asarray(ppr_idx)
    ppr_scores = np.asarray(ppr_scores)

    idx_pad = np.zeros((N_CORES, ROWS_PAD, K), dtype=np.int64)
    sc_pad = np.zeros((N_CORES, ROWS_PAD, K), dtype=np.float32)
    # spread zero-weight padding rows' indices across chunks; they dedup to
    # one slot per segment
    idx_pad[:, ROWS_PER_CORE:] = (np.arange(K) % N_CHUNKS) * CHUNK
    idx_pad[:, :ROWS_PER_CORE] = ppr_idx.reshape(N_CORES, ROWS_PER_CORE, K)
    sc_pad[:, :ROWS_PER_CORE] = ppr_scores.reshape(N_CORES, ROWS_PER_CORE, K)

    in_maps = []
    spills = []
    for c in range(N_CORES):
        idx16, wmat, spill = _prep_core_inputs(idx_pad[c], sc_pad[c])
        in_maps.append({"x": x16, "idx16": idx16, "wmat": wmat})
        spills.append(spill)
    return in_maps, spills


def make_in_maps(x, ppr_idx, ppr_scores):
    in_maps, _ = _prep_all(x, ppr_idx, ppr_scores)
    return in_maps


def kernel(x, ppr_idx, ppr_scores):
    from concourse.bass_utils import run_bass_kernel_spmd

    nc = _build_program()
    x = np.asarray(x)
    in_maps, spills = _prep_all(x, ppr_idx, ppr_scores)
    res = run_bass_kernel_spmd(nc, in_maps, core_ids=list(range(N_CORES)))
    out = np.concatenate(
        [res.results[c]["out"][:ROWS_PER_CORE] for c in range(N_CORES)], axis=0
    ).astype(np.float32)
    # host-side correction for rare segment-capacity spills
    for c in range(N_CORES):
        for local_row, src_row, score in spills[c]:
            if local_row < ROWS_PER_CORE:
                out[c * ROWS_PER_CORE + local_row] += score * x[src_row]
    return out
